# revision 36
# baseline (speedup 1.0000x reference)
"""Trainium2 Bass kernel for ChemicalNet (per-species MLP / MoE routing).

Strategy
--------
Only atoms whose species is in {1, 6, 7, 8} produce output (others are 0),
and each such atom only needs ITS OWN species' 3-layer MLP.  The reference
runs all 4 expert networks on all atoms; we route on the host instead:

- host: map species -> expert index, collect per-expert atom index lists
- shard: 2 cores per expert, each core gets half of that expert's atoms
  (the per-core in_map carries that expert's weights, so the single SPMD
  program is expert-agnostic)
- host passes the gathered embedding columns TRANSPOSED ([128, n]) in
  fp16: no device transposes, half the DMA bytes of fp32, full PE rate
  (1 col/cycle), fast weight load (so LDWEIGHTS hides behind matmuls,
  unlike f32r), and ~1e-3 worst-case relative error -- well inside the
  2e-2 gate.  PSUM accumulation stays fp32.
- device: per chunk (<= 512 atoms, one PSUM bank): L1 = 2 matmuls into a
  [128, 1024] PSUM tile, ONE SiLU ACTIVATE over both halves (the scalar
  engine costs ~218ns fixed per ACTIVATE; fewer/bigger is faster), L2 =
  4 accumulating matmuls + one ACTIVATE, L3 accumulates [1, f] into a
  corner of the L2 PSUM tile after its ACTIVATE has read it (WAR handled
  by Tile), DVE copies the row into a staging [1, n] tile
- the scalar SILU chain is the bottleneck (4 cols/atom at 1.2 GHz ~=
  9.3us/core) -- everything else is scheduled to keep it gapless:
  ramped first chunk so the chain starts as soon as the first DMA piece
  lands, a small last chunk so the post-chain L3+copy+DMA tail is short
- DMA plan: ONE DRAM tensor [w1 | emb-chunk0 | w2 | w3 | emb-rest]; the
  first piece delivers weights + chunk 0 together; each dma_start costs
  ~680ns descriptor-gen on its sequencer, so pieces are few and
  deadline-ordered, all on the sync HWDGE queue.  No DMAs ride the
  scalar queue, so walrus's ACT_TABLE_LOAD (no data deps) runs during
  the DMA-in window and the first SILU fires the moment L1(c0) lands.
  Output: a bulk DMA fired one chunk early on sync + a small final DMA
  on the (by then idle) scalar queue.
- a few zero matmuls run during the DMA-in window so the PE's HAM clock
  gate (1.2 GHz cold -> 2.4 GHz after ~3.4us of activity) flips before
  the real matmul stream arrives
- host scatters the compact per-core [1, n] outputs back to [N, 1]

Biases in this problem are identically zero (host-verified); nonzero
biases take a per-half ACTIVATE path with per-partition bias APs.
"""

import numpy as np

import concourse.bass as bass
import concourse.tile as tile
from concourse import bacc, mybir
from concourse.bass_utils import run_bass_kernel_spmd

N_CORES = 8
NSPECIES = 4
SPECIES_Z = np.array([1, 6, 7, 8], dtype=np.int32)
MAXIDX = 118
D = 128          # embedding dim
H = 256          # hidden dim
F = 512          # atom-chunk size (one PSUM bank of fp32)
FP = mybir.dt.float32
SILU = mybir.ActivationFunctionType.Silu
N_WARM_MM = 6
WCOLS = 770      # packed weight cols: w1 256 | w2 512 | w3 2


def _chunk_sizes(npad):
    """Ramped first chunks, small last chunk, 512s in between."""
    sizes = []
    for s in (128, 256):
        if sum(sizes) + s <= npad:
            sizes.append(s)
    tail = 128 if npad - sum(sizes) > 2 * F else 0
    while npad - sum(sizes) - tail > F:
        sizes.append(F)
    if npad - sum(sizes) - tail:
        sizes.append(npad - sum(sizes) - tail)
    if tail:
        sizes.append(tail)
    return sizes


def _build_program(npad, zero_bias, mmdt):
    nc = bacc.Bacc("TRN2", target_bir_lowering=False, debug=False,
                   num_devices=N_CORES)

    # layout: [w1 (256) | emb chunk0 | w2 blocks (512) | w3 (2) | emb rest]
    # so the first DMA piece = weights + chunk 0.  Atom a >= c0size lives
    # at col WCOLS + a.
    x_d = nc.dram_tensor("x", [D, WCOLS + npad], mmdt, kind="ExternalInput")
    if not zero_bias:
        b1_d = nc.dram_tensor("b1", [128, 2], FP, kind="ExternalInput")
        b2_d = nc.dram_tensor("b2", [128, 2], FP, kind="ExternalInput")
        b3_d = nc.dram_tensor("b3", [1, 1], FP, kind="ExternalInput")
    out_d = nc.dram_tensor("out", [1, npad], FP, kind="ExternalOutput")

    sizes = _chunk_sizes(npad)
    chunks = []
    c0 = 0
    for s in sizes:
        chunks.append((c0, s))
        c0 += s
    nch = len(chunks)
    C0 = chunks[0][1]          # first-chunk atom count (lives inside x0)
    X0 = 256 + C0 + 514        # piece 0: w1 | emb c0 | w2 | w3

    # DMA pieces (x_d cols): p0 = w + c0, then per chunk through the
    # ramp, then two-chunk pieces
    bounds = []
    for i in (1, 2, 3):
        if i < nch:
            bounds.append(chunks[i][0])
    i = 5
    while i < nch:
        bounds.append(chunks[i][0])
        i += 2
    # piece 0 carries only what L1(c0) reads (w1 + chunk-0 embedding);
    # w2/w3 follow as piece 1, needed ~1.2us later by L2(c0)
    pieces = [(0, 256 + C0), (256 + C0, X0)]
    for j, b in enumerate(bounds):
        e = bounds[j + 1] if j + 1 < len(bounds) else npad
        if e > b:
            pieces.append((WCOLS + b, WCOLS + e))

    with tile.TileContext(nc) as tc:
        with (
            tc.tile_pool(name="singles", bufs=1) as singles,
            tc.tile_pool(name="z1p", bufs=3) as z1p,
            tc.tile_pool(name="z2p", bufs=3) as z2p,
            tc.tile_pool(name="ps", bufs=4, space="PSUM") as psp,
        ):
            const_t = singles.tile([128, F], mmdt)
            nc.vector.memset(const_t[:], 0.0)

            # all input DMAs on the sync HWDGE queue in deadline order;
            # x_t holds the whole packed image, tracked by Tile
            x_t = singles.tile([D, WCOLS + npad], mmdt)
            for pb, pe in pieces:
                nc.sync.dma_start(x_t[:, pb:pe], x_d[:, pb:pe])
            if not zero_bias:
                b1_t = singles.tile([128, 2], FP)
                nc.sync.dma_start(b1_t[:], b1_d[:])
                b2_t = singles.tile([128, 2], FP)
                nc.sync.dma_start(b2_t[:], b2_d[:])
                b3_t = singles.tile([1, 1], FP)
                nc.sync.dma_start(b3_t[:], b3_d[:])

            out_t = singles.tile([1, npad], FP)

            # HAM warm-up: zero matmuls with no input dependencies keep
            # the PE busy during the DMA-in window so the clock gate is
            # at 2.4 GHz when the real matmul stream starts.
            ps_warm = psp.tile([128, 2 * F], FP, tag="ps", name="warm")
            for _ in range(N_WARM_MM):
                nc.tensor.matmul(ps_warm[:, :F], const_t[:, :128],
                                 const_t[:, :F], start=True, stop=True)

            def wcols(a, b):
                # logical weight col -> physical x col (emb c0 sits at
                # [256 : 256+C0], shifting w2/w3 up by C0)
                return x_t[:, a:b] if a < 256 else x_t[:, a + C0:b + C0]

            def emb_cols(c0_, f):
                if c0_ < C0:
                    return x_t[:, 256 + c0_:256 + c0_ + f]
                return x_t[:, WCOLS + c0_:WCOLS + c0_ + f]

            def m_off(f):
                # matmul output must stay inside one 512-col PSUM bank:
                # pack the m1 half right after m0 only when both fit bank 0
                return f if 2 * f <= F else F

            def act_pair(z_t, ps_t, f, b_t):
                """SiLU both m-halves of a psum tile -> z SBUF (one
                ACTIVATE in the zero-bias case)."""
                off = m_off(f)
                if zero_bias:
                    nc.scalar.activation(z_t[:, :off + f], ps_t[:, :off + f],
                                         SILU)
                else:
                    for m in range(2):
                        nc.scalar.activation(
                            z_t[:, m * off:m * off + f],
                            ps_t[:, m * off:m * off + f], SILU,
                            bias=b_t[:, m:m + 1])

            z1s, z2s, ps2s = {}, {}, {}

            def emit_l1(ci):
                c0_, f = chunks[ci]
                ps1 = psp.tile([128, 2 * F], FP, tag="ps", name=f"ps1_{ci}")
                off = m_off(f)
                for m in range(2):
                    nc.tensor.matmul(ps1[:, m * off:m * off + f],
                                     wcols(m * 128, (m + 1) * 128),
                                     emb_cols(c0_, f), start=True, stop=True)
                z1 = z1p.tile([128, 2 * F], mmdt, tag="z1", name=f"z1_{ci}")
                act_pair(z1, ps1, f, None if zero_bias else b1_t)
                z1s[ci] = z1

            def emit_l2(ci):
                c0_, f = chunks[ci]
                z1 = z1s[ci]
                off = m_off(f)
                ps2 = psp.tile([128, 2 * F], FP, tag="ps", name=f"ps2_{ci}")
                for m in range(2):
                    for k in range(2):
                        wcol = 256 + 128 * (2 * m + k)
                        nc.tensor.matmul(
                            ps2[:, m * off:m * off + f],
                            wcols(wcol, wcol + 128),
                            z1[:, k * off:k * off + f],
                            start=(k == 0), stop=(k == 1))
                z2 = z2p.tile([128, 2 * F], mmdt, tag="z2", name=f"z2_{ci}")
                act_pair(z2, ps2, f, None if zero_bias else b2_t)
                z2s[ci], ps2s[ci] = z2, ps2

            def emit_l3(ci):
                c0_, f = chunks[ci]
                z2 = z2s[ci]
                off = m_off(f)
                # L3 accumulates into a corner of ps2 after its ACTIVATE
                # has read it (WAR handled by Tile) -- no extra PSUM bank.
                ps3 = ps2s[ci][0:1, 0:f]
                nc.tensor.matmul(ps3, wcols(768, 769), z2[:, :f],
                                 start=True, stop=False)
                nc.tensor.matmul(ps3, wcols(769, 770), z2[:, off:off + f],
                                 start=False, stop=True)
                if zero_bias:
                    nc.vector.tensor_copy(out_t[0:1, c0_:c0_ + f], ps3)
                else:
                    nc.vector.tensor_scalar_add(out_t[0:1, c0_:c0_ + f], ps3,
                                                b3_t[0:1, 0:1])

            # Software-pipelined emission: L1 runs three chunks ahead of
            # L2, L3 one behind, so the scalar engine's in-order ACTIVATE
            # queue never head-of-line blocks and the PE always has
            # independent matmuls queued.
            depth = min(3, nch)
            for ci in range(depth):
                emit_l1(ci)
            for ci in range(nch):
                emit_l2(ci)
                if ci + depth < nch:
                    emit_l1(ci + depth)
                if ci >= 1:
                    emit_l3(ci - 1)
                    if ci - 1 == nch - 2:
                        # bulk output DMA: descriptor-gen (~680ns) and
                        # completion overlap the last chunk's compute
                        lb = chunks[nch - 1][0]
                        nc.sync.dma_start(out_d[:, :lb], out_t[:, :lb])
            emit_l3(nch - 1)
            if nch > 1:
                # final piece on the scalar queue: idle after the last
                # ACTIVATE, so this gen doesn't serialize behind the bulk
                lb = chunks[nch - 1][0]
                nc.scalar.dma_start(out_d[:, lb:], out_t[:, lb:])
            else:
                nc.sync.dma_start(out_d[:], out_t[:])

    nc.compile()
    return nc


def _route(species):
    """species values -> expert idx (-1 unknown); per-core row assignments."""
    conv = np.full(MAXIDX + 2, -1, dtype=np.int32)
    conv[SPECIES_Z] = np.arange(NSPECIES, dtype=np.int32)
    idx = conv[species]
    core_rows = []
    for s in range(NSPECIES):
        rows = np.flatnonzero(idx == s)
        h = (len(rows) + 1) // 2
        core_rows.append(rows[:h])
        core_rows.append(rows[h:])
    return core_rows


def _run(inputs, trace=False, use_f32r=False, use_bf16=False):
    species = inputs["species"]
    embedding = np.ascontiguousarray(inputs["embedding"], dtype=np.float32)
    n_atoms = species.shape[0]
    out_full = np.zeros((n_atoms, 1), dtype=np.float32)

    core_rows = _route(np.asarray(species))
    nmax = max(len(r) for r in core_rows)
    if nmax == 0:
        return out_full, None
    npad = -(-nmax // 4) * 4

    zero_bias = all(
        not np.any(np.asarray(inputs[k])) for k in ("b1", "b2", "b3"))
    if use_bf16:
        mmdt = mybir.dt.bfloat16
    elif use_f32r:
        mmdt = mybir.dt.float32r
    else:
        mmdt = mybir.dt.float16
    np_mm = mybir.dt.np(mmdt)
    nc = _build_program(npad, zero_bias, mmdt)
    C0 = _chunk_sizes(npad)[0]

    in_maps = []
    for c in range(N_CORES):
        s = c // 2
        rows = core_rows[c]
        # layout: [w1 | emb c0 (C0) | w2 | w3 | emb rest]
        x = np.zeros((D, WCOLS + npad), dtype=np_mm)
        embT = embedding[rows].T.astype(np_mm) if len(rows) else \
            np.zeros((D, 0), dtype=np_mm)
        ne = embT.shape[1]
        x[:, 256:256 + min(ne, C0)] = embT[:, :C0]
        if ne > C0:
            x[:, WCOLS + C0:WCOLS + ne] = embT[:, C0:]
        w1 = np.asarray(inputs["W1"][s], dtype=np.float32)      # [128, 256]
        w2 = np.asarray(inputs["W2"][s], dtype=np.float32)      # [256, 256]
        w3 = np.asarray(inputs["W3"][s], dtype=np.float32)      # [256, 1]
        x[:, 0:256] = w1.astype(np_mm)
        for m in range(2):
            for k in range(2):
                col = 256 + C0 + 128 * (2 * m + k)
                x[:, col:col + 128] = \
                    w2[k * 128:(k + 1) * 128,
                       m * 128:(m + 1) * 128].astype(np_mm)
        x[:, 256 + C0 + 512] = w3[0:128, 0].astype(np_mm)
        x[:, 256 + C0 + 513] = w3[128:256, 0].astype(np_mm)
        im = {"x": x}
        if not zero_bias:
            im["b1"] = np.ascontiguousarray(
                np.asarray(inputs["b1"][s], dtype=np.float32).reshape(2, 128).T)
            im["b2"] = np.ascontiguousarray(
                np.asarray(inputs["b2"][s], dtype=np.float32).reshape(2, 128).T)
            im["b3"] = np.asarray(inputs["b3"][s],
                                  dtype=np.float32).reshape(1, 1)
        in_maps.append(im)

    res = run_bass_kernel_spmd(nc, in_maps, core_ids=list(range(N_CORES)),
                               trace=trace)
    for c in range(N_CORES):
        rows = core_rows[c]
        if len(rows):
            out_full[rows, 0] = res.results[c]["out"][0, :len(rows)]
    return out_full, res


def kernel(**inputs) -> np.ndarray:
    out, _ = _run(inputs, trace=False)
    return out


# revision 37
# speedup vs baseline: 1.0175x; 1.0175x over previous
"""Trainium2 Bass kernel for ChemicalNet (per-species MLP / MoE routing).

Strategy
--------
Only atoms whose species is in {1, 6, 7, 8} produce output (others are 0),
and each such atom only needs ITS OWN species' 3-layer MLP.  The reference
runs all 4 expert networks on all atoms; we route on the host instead:

- host: map species -> expert index, collect per-expert atom index lists
- shard: 2 cores per expert, each core gets half of that expert's atoms
  (the per-core in_map carries that expert's weights, so the single SPMD
  program is expert-agnostic)
- host passes the gathered embedding columns TRANSPOSED ([128, n]) in
  fp16: no device transposes, half the DMA bytes of fp32, full PE rate
  (1 col/cycle), fast weight load (so LDWEIGHTS hides behind matmuls,
  unlike f32r), and ~1e-3 worst-case relative error -- well inside the
  2e-2 gate.  PSUM accumulation stays fp32.
- device: per chunk (<= 512 atoms, one PSUM bank): L1 = 2 matmuls into a
  [128, 1024] PSUM tile, ONE SiLU ACTIVATE over both halves (the scalar
  engine costs ~218ns fixed per ACTIVATE; fewer/bigger is faster), L2 =
  4 accumulating matmuls + one ACTIVATE, L3 accumulates [1, f] into a
  corner of the L2 PSUM tile after its ACTIVATE has read it (WAR handled
  by Tile), DVE copies the row into a staging [1, n] tile
- the scalar SILU chain is the bottleneck (4 cols/atom at 1.2 GHz ~=
  9.3us/core) -- everything else is scheduled to keep it gapless:
  ramped first chunk so the chain starts as soon as the first DMA piece
  lands, a small last chunk so the post-chain L3+copy+DMA tail is short
- DMA plan: ONE DRAM tensor [w1 | emb-chunk0 | w2 | w3 | emb-rest]; the
  first piece delivers weights + chunk 0 together; each dma_start costs
  ~680ns descriptor-gen on its sequencer, so pieces are few and
  deadline-ordered, all on the sync HWDGE queue.  No DMAs ride the
  scalar queue, so walrus's ACT_TABLE_LOAD (no data deps) runs during
  the DMA-in window and the first SILU fires the moment L1(c0) lands.
  Output: a bulk DMA fired one chunk early on sync + a small final DMA
  on the (by then idle) scalar queue.
- a few zero matmuls run during the DMA-in window so the PE's HAM clock
  gate (1.2 GHz cold -> 2.4 GHz after ~3.4us of activity) flips before
  the real matmul stream arrives
- host scatters the compact per-core [1, n] outputs back to [N, 1]

Biases in this problem are identically zero (host-verified); nonzero
biases take a per-half ACTIVATE path with per-partition bias APs.
"""

import numpy as np

import concourse.bass as bass
import concourse.tile as tile
from concourse import bacc, mybir
from concourse.bass_utils import run_bass_kernel_spmd

N_CORES = 8
NSPECIES = 4
SPECIES_Z = np.array([1, 6, 7, 8], dtype=np.int32)
MAXIDX = 118
D = 128          # embedding dim
H = 256          # hidden dim
F = 512          # atom-chunk size (one PSUM bank of fp32)
FP = mybir.dt.float32
SILU = mybir.ActivationFunctionType.Silu
N_WARM_MM = 6
WCOLS = 770      # packed weight cols: w1 256 | w2 512 | w3 2


def _chunk_sizes(npad):
    """Ramped first chunks, small last chunk, 512s in between."""
    sizes = []
    for s in (128, 256):
        if sum(sizes) + s <= npad:
            sizes.append(s)
    tail = 128 if npad - sum(sizes) > 2 * F else 0
    while npad - sum(sizes) - tail > F:
        sizes.append(F)
    if npad - sum(sizes) - tail:
        sizes.append(npad - sum(sizes) - tail)
    if tail:
        sizes.append(tail)
    return sizes


def _build_program(npad, zero_bias, mmdt):
    nc = bacc.Bacc("TRN2", target_bir_lowering=False, debug=False,
                   num_devices=N_CORES)

    # layout: [w1 (256) | emb chunk0 | w2 blocks (512) | w3 (2) | emb rest]
    # so the first DMA piece = weights + chunk 0.  Atom a >= c0size lives
    # at col WCOLS + a.
    x_d = nc.dram_tensor("x", [D, WCOLS + npad], mmdt, kind="ExternalInput")
    if not zero_bias:
        b1_d = nc.dram_tensor("b1", [128, 2], FP, kind="ExternalInput")
        b2_d = nc.dram_tensor("b2", [128, 2], FP, kind="ExternalInput")
        b3_d = nc.dram_tensor("b3", [1, 1], FP, kind="ExternalInput")
    out_d = nc.dram_tensor("out", [1, npad], FP, kind="ExternalOutput")

    sizes = _chunk_sizes(npad)
    chunks = []
    c0 = 0
    for s in sizes:
        chunks.append((c0, s))
        c0 += s
    nch = len(chunks)
    C0 = chunks[0][1]          # first-chunk atom count (lives inside x0)
    X0 = 256 + C0 + 514        # piece 0: w1 | emb c0 | w2 | w3

    # DMA pieces (x_d cols): p0 = w + c0, then per chunk through the
    # ramp, then two-chunk pieces
    bounds = []
    for i in (1, 2, 3):
        if i < nch:
            bounds.append(chunks[i][0])
    i = 5
    while i < nch:
        bounds.append(chunks[i][0])
        i += 2
    pieces = [(0, X0)]
    for j, b in enumerate(bounds):
        e = bounds[j + 1] if j + 1 < len(bounds) else npad
        if e > b:
            pieces.append((WCOLS + b, WCOLS + e))

    with tile.TileContext(nc) as tc:
        with (
            tc.tile_pool(name="singles", bufs=1) as singles,
            tc.tile_pool(name="z1p", bufs=3) as z1p,
            tc.tile_pool(name="z2p", bufs=3) as z2p,
            tc.tile_pool(name="ps", bufs=4, space="PSUM") as psp,
        ):
            const_t = singles.tile([128, F], mmdt)
            nc.vector.memset(const_t[:], 0.0)

            # all input DMAs on the sync HWDGE queue in deadline order;
            # x_t holds the whole packed image, tracked by Tile
            x_t = singles.tile([D, WCOLS + npad], mmdt)
            for pb, pe in pieces:
                nc.sync.dma_start(x_t[:, pb:pe], x_d[:, pb:pe])
            if not zero_bias:
                b1_t = singles.tile([128, 2], FP)
                nc.sync.dma_start(b1_t[:], b1_d[:])
                b2_t = singles.tile([128, 2], FP)
                nc.sync.dma_start(b2_t[:], b2_d[:])
                b3_t = singles.tile([1, 1], FP)
                nc.sync.dma_start(b3_t[:], b3_d[:])

            out_t = singles.tile([1, npad], FP)

            # HAM warm-up: zero matmuls with no input dependencies keep
            # the PE busy during the DMA-in window so the clock gate is
            # at 2.4 GHz when the real matmul stream starts.
            ps_warm = psp.tile([128, 2 * F], FP, tag="ps", name="warm")
            for _ in range(N_WARM_MM):
                nc.tensor.matmul(ps_warm[:, :F], const_t[:, :128],
                                 const_t[:, :F], start=True, stop=True)

            def wcols(a, b):
                # logical weight col -> physical x col (emb c0 sits at
                # [256 : 256+C0], shifting w2/w3 up by C0)
                return x_t[:, a:b] if a < 256 else x_t[:, a + C0:b + C0]

            def emb_cols(c0_, f):
                if c0_ < C0:
                    return x_t[:, 256 + c0_:256 + c0_ + f]
                return x_t[:, WCOLS + c0_:WCOLS + c0_ + f]

            def m_off(f):
                # matmul output must stay inside one 512-col PSUM bank:
                # pack the m1 half right after m0 only when both fit bank 0
                return f if 2 * f <= F else F

            def act_pair(z_t, ps_t, f, b_t):
                """SiLU both m-halves of a psum tile -> z SBUF (one
                ACTIVATE in the zero-bias case)."""
                off = m_off(f)
                if zero_bias:
                    nc.scalar.activation(z_t[:, :off + f], ps_t[:, :off + f],
                                         SILU)
                else:
                    for m in range(2):
                        nc.scalar.activation(
                            z_t[:, m * off:m * off + f],
                            ps_t[:, m * off:m * off + f], SILU,
                            bias=b_t[:, m:m + 1])

            z1s, z2s, ps2s = {}, {}, {}

            def emit_l1(ci):
                c0_, f = chunks[ci]
                ps1 = psp.tile([128, 2 * F], FP, tag="ps", name=f"ps1_{ci}")
                off = m_off(f)
                for m in range(2):
                    nc.tensor.matmul(ps1[:, m * off:m * off + f],
                                     wcols(m * 128, (m + 1) * 128),
                                     emb_cols(c0_, f), start=True, stop=True)
                z1 = z1p.tile([128, 2 * F], mmdt, tag="z1", name=f"z1_{ci}")
                act_pair(z1, ps1, f, None if zero_bias else b1_t)
                z1s[ci] = z1

            def emit_l2(ci):
                c0_, f = chunks[ci]
                z1 = z1s[ci]
                off = m_off(f)
                ps2 = psp.tile([128, 2 * F], FP, tag="ps", name=f"ps2_{ci}")
                for m in range(2):
                    for k in range(2):
                        wcol = 256 + 128 * (2 * m + k)
                        nc.tensor.matmul(
                            ps2[:, m * off:m * off + f],
                            wcols(wcol, wcol + 128),
                            z1[:, k * off:k * off + f],
                            start=(k == 0), stop=(k == 1))
                z2 = z2p.tile([128, 2 * F], mmdt, tag="z2", name=f"z2_{ci}")
                act_pair(z2, ps2, f, None if zero_bias else b2_t)
                z2s[ci], ps2s[ci] = z2, ps2

            def emit_l3(ci):
                c0_, f = chunks[ci]
                z2 = z2s[ci]
                off = m_off(f)
                # L3 accumulates into a corner of ps2 after its ACTIVATE
                # has read it (WAR handled by Tile) -- no extra PSUM bank.
                ps3 = ps2s[ci][0:1, 0:f]
                nc.tensor.matmul(ps3, wcols(768, 769), z2[:, :f],
                                 start=True, stop=False)
                nc.tensor.matmul(ps3, wcols(769, 770), z2[:, off:off + f],
                                 start=False, stop=True)
                if zero_bias:
                    nc.vector.tensor_copy(out_t[0:1, c0_:c0_ + f], ps3)
                else:
                    nc.vector.tensor_scalar_add(out_t[0:1, c0_:c0_ + f], ps3,
                                                b3_t[0:1, 0:1])

            # Software-pipelined emission: L1 runs three chunks ahead of
            # L2, L3 one behind, so the scalar engine's in-order ACTIVATE
            # queue never head-of-line blocks and the PE always has
            # independent matmuls queued.
            depth = min(3, nch)
            for ci in range(depth):
                emit_l1(ci)
            for ci in range(nch):
                emit_l2(ci)
                if ci + depth < nch:
                    emit_l1(ci + depth)
                if ci >= 1:
                    emit_l3(ci - 1)
                    if ci - 1 == nch - 2:
                        # bulk output DMA: descriptor-gen (~680ns) and
                        # completion overlap the last chunk's compute
                        lb = chunks[nch - 1][0]
                        nc.sync.dma_start(out_d[:, :lb], out_t[:, :lb])
            emit_l3(nch - 1)
            if nch > 1:
                # final piece on the scalar queue: idle after the last
                # ACTIVATE, so this gen doesn't serialize behind the bulk
                lb = chunks[nch - 1][0]
                nc.scalar.dma_start(out_d[:, lb:], out_t[:, lb:])
            else:
                nc.sync.dma_start(out_d[:], out_t[:])

    nc.compile()
    return nc


def _route(species):
    """species values -> expert idx (-1 unknown); per-core row assignments."""
    conv = np.full(MAXIDX + 2, -1, dtype=np.int32)
    conv[SPECIES_Z] = np.arange(NSPECIES, dtype=np.int32)
    idx = conv[species]
    core_rows = []
    for s in range(NSPECIES):
        rows = np.flatnonzero(idx == s)
        h = (len(rows) + 1) // 2
        core_rows.append(rows[:h])
        core_rows.append(rows[h:])
    return core_rows


def _run(inputs, trace=False, use_f32r=False, use_bf16=False):
    species = inputs["species"]
    embedding = np.ascontiguousarray(inputs["embedding"], dtype=np.float32)
    n_atoms = species.shape[0]
    out_full = np.zeros((n_atoms, 1), dtype=np.float32)

    core_rows = _route(np.asarray(species))
    nmax = max(len(r) for r in core_rows)
    if nmax == 0:
        return out_full, None
    npad = -(-nmax // 4) * 4

    zero_bias = all(
        not np.any(np.asarray(inputs[k])) for k in ("b1", "b2", "b3"))
    if use_bf16:
        mmdt = mybir.dt.bfloat16
    elif use_f32r:
        mmdt = mybir.dt.float32r
    else:
        mmdt = mybir.dt.float16
    np_mm = mybir.dt.np(mmdt)
    nc = _build_program(npad, zero_bias, mmdt)
    C0 = _chunk_sizes(npad)[0]

    in_maps = []
    for c in range(N_CORES):
        s = c // 2
        rows = core_rows[c]
        # layout: [w1 | emb c0 (C0) | w2 | w3 | emb rest]
        x = np.zeros((D, WCOLS + npad), dtype=np_mm)
        embT = embedding[rows].T.astype(np_mm) if len(rows) else \
            np.zeros((D, 0), dtype=np_mm)
        ne = embT.shape[1]
        x[:, 256:256 + min(ne, C0)] = embT[:, :C0]
        if ne > C0:
            x[:, WCOLS + C0:WCOLS + ne] = embT[:, C0:]
        w1 = np.asarray(inputs["W1"][s], dtype=np.float32)      # [128, 256]
        w2 = np.asarray(inputs["W2"][s], dtype=np.float32)      # [256, 256]
        w3 = np.asarray(inputs["W3"][s], dtype=np.float32)      # [256, 1]
        x[:, 0:256] = w1.astype(np_mm)
        for m in range(2):
            for k in range(2):
                col = 256 + C0 + 128 * (2 * m + k)
                x[:, col:col + 128] = \
                    w2[k * 128:(k + 1) * 128,
                       m * 128:(m + 1) * 128].astype(np_mm)
        x[:, 256 + C0 + 512] = w3[0:128, 0].astype(np_mm)
        x[:, 256 + C0 + 513] = w3[128:256, 0].astype(np_mm)
        im = {"x": x}
        if not zero_bias:
            im["b1"] = np.ascontiguousarray(
                np.asarray(inputs["b1"][s], dtype=np.float32).reshape(2, 128).T)
            im["b2"] = np.ascontiguousarray(
                np.asarray(inputs["b2"][s], dtype=np.float32).reshape(2, 128).T)
            im["b3"] = np.asarray(inputs["b3"][s],
                                  dtype=np.float32).reshape(1, 1)
        in_maps.append(im)

    res = run_bass_kernel_spmd(nc, in_maps, core_ids=list(range(N_CORES)),
                               trace=trace)
    for c in range(N_CORES):
        rows = core_rows[c]
        if len(rows):
            out_full[rows, 0] = res.results[c]["out"][0, :len(rows)]
    return out_full, res


def kernel(**inputs) -> np.ndarray:
    out, _ = _run(inputs, trace=False)
    return out
